# revision 88
# baseline (speedup 1.0000x reference)
"""GCN (6-layer: conv1 + 4x shared conv2 + mean-pool + linear) on 8 Trainium2
NeuronCores via Bass/Tile.

Strategy (dst-sharded message passing with a replicated gather table):
  - Nodes are sharded contiguously across cores (NPC = N/C per core).
  - Per conv layer: every core transforms its own nodes (h @ W), scales rows
    by dis = deg^-1/2 (GCN norm factorization: enorm = dis[src]*dis[dst], so
    agg[d] = dis[d] * sum_e t[src_e] with t = dis*hw, self term = dis[d]*t[d]).
    The table is split in two halves aligned to window-group completion and
    double-buffered by layer parity: each half is AllGather'd as soon as its
    groups' transforms finish, so the first half's exchange overlaps the
    second half of the layer and the second half's exchange overlaps the
    next layer's first gathers. No serial exchange gap remains.
  - Each core gathers its incident edges' source rows with dma_gather,
    strictly rotating over the 4 SWDGE queues so all 8 Pool-engine Q7 CPUs
    generate descriptors concurrently (4x desc-gen throughput; this is the
    kernel's critical path). Rows are 256B-strided but only the 128B of
    payload is transferred (elem 64 x bf16; the stock 256B-elem assert is
    bypassed - HW only needs the row stride to be a 256B multiple), halving
    DMA-ring traffic. Bucket edges are sorted by source row; partial chunks
    sit at call tails and a static per-call num_idxs trim skips the padding
    descriptors entirely.
  - Aggregation: per 128-edge chunk, a one-hot S matrix (built on the DVE
    from a dst-local id table with is_equal against an iota) is the
    stationary matmul operand against the gathered rows, PSUM-accumulated
    per 16-window group with lazy 2KB-bank zeroing, then drained
    (+self term, *dis, +bias, relu) and re-transformed per group.
  - Mean-pool partial sums per graph use the same one-hot matmul trick;
    per-core partials are combined on the host with the final tiny linear.

The single SPMD program is identical on all cores: all per-core variation
travels through input tensors; chunk/bucket counts are padded to the max
over cores so the instruction stream is uniform.
"""

import math
import sys

sys.path.insert(0, "/opt/trn_rl_repo")

import numpy as np
import ml_dtypes

import concourse.bass as bass
import concourse.mybir as mybir
import concourse.tile as tile
from concourse import bacc
from concourse.masks import make_identity

BF16 = mybir.dt.bfloat16
F32 = mybir.dt.float32
I16 = mybir.dt.int16
FP8 = mybir.dt.float8e4
ALU = mybir.AluOpType

NP_BF16 = ml_dtypes.bfloat16
NP_FP8 = ml_dtypes.float8_e4m3

CALLCH = 8         # chunks per dma_gather call (1024 idxs = HW packet limit)
WGRP = 16          # dst windows per PSUM accumulation group
BANKW = 8          # windows per PSUM 2KB zero-region (bank)


def _ap3(ap, pattern, offset=None):
    """Hand-build a broadcast/strided AP on the same tensor."""
    return bass.AP(ap.tensor, ap.offset if offset is None else offset, pattern)


def dma_gather_raw(nc, out_ap, in_ap, idxs_ap, num_idxs, elem_size,
                   elem_step, queue_num):
    """dma_gather allowing elem 128B at a 256B row stride (the stock wrapper
    asserts elem%256B, but HW only needs the stride to be a 256B multiple)."""
    gp = nc.gpsimd
    gp._assert_queue_num(queue_num)
    assert idxs_ap.dtype == mybir.dt.int16
    assert in_ap.dtype == out_ap.dtype
    from concourse.bass import MemorySpace
    assert in_ap.space == MemorySpace.DRAM
    assert idxs_ap.space == MemorySpace.SBUF
    assert out_ap.space == MemorySpace.SBUF
    assert in_ap.ap[0][0] == elem_step
    stride_bytes = elem_step * mybir.dt.size(in_ap.dtype)
    stride_bytes_256 = stride_bytes // 256
    assert stride_bytes % 256 == 0 and stride_bytes_256 < 256
    _in_ap = gp.lower_ap_dma(in_ap, for_custom_bir_dma=True)
    _idxs_ap = gp.lower_ap(idxs_ap)
    _out_ap = gp.lower_ap(out_ap)
    return gp.add_instruction(
        mybir.InstDMAGatherAnt(
            name=nc.get_next_instruction_name(),
            ins=[*_in_ap, _idxs_ap,
                 gp.lower_val_access(gp.to_reg(num_idxs))],
            outs=[_out_ap],
            transpose=False,
            num_idxs=num_idxs,
            elem_size=elem_size,
            stride_bytes_256=stride_bytes_256,
            gen_mode=0,
            single_packet=True,
            queue_num=queue_num,
        ))


# ---------------------------------------------------------------------------
# Host preprocessing
# ---------------------------------------------------------------------------

def prep(x, W1, b1, W2, b2, Wl, bl, edge_index, batch, C, G):
    x = np.asarray(x, np.float32)
    W1 = np.asarray(W1, np.float32); b1 = np.asarray(b1, np.float32)
    W2 = np.asarray(W2, np.float32); b2 = np.asarray(b2, np.float32)
    Wl = np.asarray(Wl, np.float32); bl = np.asarray(bl, np.float32)
    edge_index = np.asarray(edge_index, np.int64)
    batch = np.asarray(batch, np.int64)

    N, F = x.shape
    E = edge_index.shape[1]
    H = W1.shape[1]
    assert N % C == 0 and C % 2 == 0
    NPC = N // C
    W = math.ceil(NPC / 128)
    NPAD = W * 128
    ROWS = C * NPAD
    HR = (C // 2) * NPAD
    assert HR <= 32768, HR
    NG = math.ceil(W / WGRP)

    src, dst = edge_index[0], edge_index[1]
    deg = 1.0 + np.bincount(dst, minlength=N).astype(np.float32)
    dis = 1.0 / np.sqrt(deg)

    n = np.arange(N)
    cb = n // NPC
    lp = n % NPC
    p_ = lp % 128
    w_ = lp // 128
    # table split in two halves aligned to window-group completion (half 0 =
    # groups 0..1, half 1 = the rest), AllGather'd separately so the first
    # half's exchange runs mid-layer. Rows are window-major within the half.
    WA = min(2 * WGRP, W)               # windows in half 0
    WH = [WA, W - WA]
    TRh = [C * WH[0] * 128, C * WH[1] * 128]
    assert max(TRh) <= 32768, TRh
    h_ = (w_ >= WA).astype(np.int64)
    srow = cb * np.array(WH)[h_] * 128 + (w_ - h_ * WA) * 128 + p_
    xcol = cb * NPAD + w_ * 128 + p_    # window-major x_fm column of node n

    # --- edge bucketing: (core, src-half, dst-window) -----------------------
    ecore = dst // NPC
    edl = dst % NPC
    ew = edl // 128             # dst window
    edloc = edl % 128           # dst-local within the window
    esh = h_[src]

    cnt = np.zeros((C, 2, W), np.int64)
    np.add.at(cnt, (ecore, esh, ew), 1)
    cmax = cnt.max(axis=0)                                 # [2, W]
    Kb = np.ceil(cmax / 128).astype(np.int64)              # [2, W] chunks
    for w in range(W):
        if Kb[:, w].sum() == 0:
            Kb[0, w] = 1

    # chunk order: (group, src-half, window); within a segment, partial
    # (padded) chunks are placed at dma_gather call tails so their pad
    # indices can be -1 (the SWDGE ucode skips trailing negatives).
    chunk_w, chunk_sh = [], []
    seg_bounds = []   # (sh, lo, hi) per (g, sh) segment
    bucket_chunks = {}   # (sh, w) -> list of chunk indices (rank order)
    # segment order: group-pairs (2k, 2k+1) emit their sh=0 segments before
    # their sh=1 segments, so the second half-table AllGather is covered by
    # sh=0 gather work; at most 2 PSUM groups are live at a time.
    seg_order = []
    for k in range(0, NG, 2):
        gs = [g for g in (k, k + 1) if g < NG]
        seg_order += [(g, 0) for g in gs] + [(g, 1) for g in gs]
    for g, sh in seg_order:
        wlo, whi = g * WGRP, min((g + 1) * WGRP, W)
        if True:
            lo = len(chunk_w)
            fulls, partials = [], []
            for w in range(wlo, whi):
                k = int(Kb[sh, w])
                if k == 0:
                    continue
                # bucket's last chunk is the (potentially) partial one
                fulls.extend((w, j) for j in range(k - 1))
                if cmax[sh, w] % 128 != 0 or cmax[sh, w] == 0:
                    partials.append((w, k - 1))
                else:
                    fulls.append((w, k - 1))
            # arrange: 7 fulls + 1 partial per call while both last
            seq = []
            fi, pi = 0, 0
            while fi < len(fulls) or pi < len(partials):
                take = min(CALLCH - 1, len(fulls) - fi)
                seq.extend(fulls[fi:fi + take]); fi += take
                room = CALLCH - take
                takep = min(room, len(partials) - pi)
                seq.extend(partials[pi:pi + takep]); pi += takep
            ful_idx, par_idx = {}, {}
            partset = set(partials)
            for (w, j) in seq:
                ci = len(chunk_w)
                if (w, j) in partset:
                    par_idx[w] = ci
                else:
                    ful_idx.setdefault(w, []).append(ci)
                chunk_w.append(w); chunk_sh.append(sh)
            for w in set(ful_idx) | set(par_idx):
                lst = ful_idx.get(w, [])
                if w in par_idx:
                    lst = lst + [par_idx[w]]   # partial takes the tail ranks
                bucket_chunks[(sh, w)] = lst
            if len(chunk_w) > lo:
                seg_bounds.append((sh, lo, len(chunk_w)))
    NCHUNK = len(chunk_w)
    chunk_w = np.array(chunk_w); chunk_sh = np.array(chunk_sh)

    # start/stop flags: first/last chunk per PSUM BANK (8 windows of 64 f32
    # = one 2KB zero region; the start bit lazily zeroes the whole bank).
    bank_of_chunk = chunk_w // BANKW
    start_f = np.zeros(NCHUNK, bool); stop_f = np.zeros(NCHUNK, bool)
    for b in np.unique(bank_of_chunk):
        idxs = np.nonzero(bank_of_chunk == b)[0]
        start_f[idxs[0]] = True; stop_f[idxs[-1]] = True

    # gather calls: slice each segment into <= CALLCH chunks
    calls = []
    for sh, lo, hi in seg_bounds:
        c0 = lo
        while c0 < hi:
            c1 = min(c0 + CALLCH, hi)
            calls.append((sh, c0, c1))
            c0 = c1
    grp_of_chunk = chunk_w // WGRP

    # --- per-core edge payloads: gather indices + dst-local slots ----------
    # Within each bucket, edges are sorted by source table row; rank r of
    # bucket (sh, w) sits in chunk bucket_chunks[(sh,w)][r//128], slot r%128.
    # rank -> flat slot position lookup per bucket:
    chunkmap = {k: np.array(v, np.int64) for k, v in bucket_chunks.items()}
    idx_all = np.zeros((C, NCHUNK * 128), np.int16)
    dl_all = np.full((C, NCHUNK * 128), 400.0, np.float32)
    used = np.zeros((C, NCHUNK * 128), bool)
    for c in range(C):
        m = ecore == c
        es, ish, iw, idl = src[m], esh[m], ew[m], edloc[m]
        key = ish * W + iw
        order = np.lexsort((srow[es], key))
        es, ish, iw, idl = es[order], ish[order], iw[order], idl[order]
        key = key[order]
        uniq, first = np.unique(key, return_index=True)
        ranks = np.arange(len(key)) - first[np.searchsorted(uniq, key)]
        pos = np.empty(len(key), np.int64)
        for k in np.unique(key):
            sel = key == k
            lst = chunkmap[(k // W, k % W)]
            r = ranks[sel]
            pos[sel] = lst[r // 128] * 128 + r % 128
        idx_all[c, pos] = srow[es].astype(np.int16)
        dl_all[c, pos] = idl
        used[c, pos] = True

    # static per-call num_idxs: trim the call's trailing pad slots (uniform
    # across cores: the max fill over cores), so the SWDGE ucode generates
    # no descriptors for them
    call_nidx = []
    for (sh, c_lo, c_hi) in calls:
        lo, hi = c_lo * 128, c_hi * 128
        nidx = 1
        for c in range(C):
            u = np.nonzero(used[c, lo:hi])[0]
            if len(u):
                nidx = max(nidx, int(u[-1]) + 1)
        call_nidx.append(nidx)

    # wrapped-16 index layout, replicated to 128 partitions
    idx16 = np.zeros((C, 128, NCHUNK * 8), np.int16)
    for c in range(C):
        wrapped = idx_all[c].reshape(NCHUNK * 8, 16).T   # [16, NCHUNK*8]
        idx16[c] = np.tile(wrapped, (8, 1))
    dstloc = np.zeros((C, 128, NCHUNK), NP_BF16)
    for c in range(C):
        dstloc[c] = dl_all[c].reshape(NCHUNK, 128).T.astype(NP_BF16)

    # --- node-side tensors --------------------------------------------------
    xfm = np.zeros((F, ROWS), np.float32)
    xfm[:, xcol] = x.T
    xfm = xfm.astype(NP_BF16)

    disALL = np.zeros((128, C * W), np.float32)
    disALL[p_, cb * W + w_] = dis
    disn = np.zeros((C, 128, W), np.float32)
    for c in range(C):
        sl = slice(c * NPC, (c + 1) * NPC)
        disn[c][p_[sl], w_[sl]] = dis[sl]

    tailp = NPC - (W - 1) * 128
    mask48 = (np.arange(128) < tailp).astype(np.float32).reshape(128, 1)

    # --- pooling ------------------------------------------------------------
    BLK = math.ceil(G / 128) + 3
    wk = [int(batch[c * NPC]) // 128 for c in range(C)]
    glocal = np.full((C, 128, W), 1.0e4, np.float32)
    for c in range(C):
        sl = slice(c * NPC, (c + 1) * NPC)
        gl = batch[sl] - 128 * wk[c]
        assert gl.min() >= 0 and gl.max() < 384, (c, gl.min(), gl.max())
        glocal[c][p_[sl], w_[sl]] = gl

    meta = dict(
        N=N, F=F, H=H, E=E, G=G, C=C, NPC=NPC, W=W, NPAD=NPAD, ROWS=ROWS,
        HR=HR, NG=NG, NCHUNK=NCHUNK, chunk_w=chunk_w, chunk_sh=chunk_sh,
        bank_of_chunk=bank_of_chunk,
        start_f=start_f, stop_f=stop_f, calls=calls, grp_of_chunk=grp_of_chunk,
        call_nidx=call_nidx, WA=WA, WH=WH, TRh=TRh,
        wk=wk, BLK=BLK, blv=float(bl.reshape(-1)[0]), tailp=tailp,
    )

    shared = dict(
        xfm=xfm,
        disALL=disALL,
        W1sb=W1.astype(NP_BF16),
        W2sb=W2.astype(NP_BF16),
        Wlsb=Wl.astype(NP_BF16),
        b1sb=np.tile(b1.reshape(1, H), (128, 1)).astype(np.float32),
        b2sb=np.tile(b2.reshape(1, H), (128, 1)).astype(np.float32),
        iota128=np.tile(np.arange(128, dtype=np.float32).reshape(1, 128),
                        (128, 1)).astype(NP_BF16),
        iotaP=np.tile(np.arange(384, dtype=np.float32).reshape(1, 384),
                      (128, 1)),
        mask48=mask48,
    )
    in_maps = []
    for c in range(C):
        m = dict(shared)
        m["xown"] = np.ascontiguousarray(
            xfm[:, c * NPAD:(c + 1) * NPAD])
        m["idx16"] = idx16[c]
        m["dstloc"] = dstloc[c]
        m["disn"] = disn[c]
        m["glocal"] = glocal[c]
        in_maps.append(m)
    return meta, in_maps


# ---------------------------------------------------------------------------
# Bass/Tile program
# ---------------------------------------------------------------------------

def build(nc, meta):
    F, H, C = meta["F"], meta["H"], meta["C"]
    W, NPAD, ROWS, HR = meta["W"], meta["NPAD"], meta["ROWS"], meta["HR"]
    NG, NCHUNK, BLK, G = meta["NG"], meta["NCHUNK"], meta["BLK"], meta["G"]
    chunk_w = meta["chunk_w"]
    start_f, stop_f = meta["start_f"], meta["stop_f"]
    calls, wk = meta["calls"], meta["wk"]
    WA, WH, TRh = meta["WA"], meta["WH"], meta["TRh"]
    rg = [list(range(C))]

    # external inputs
    xfm_e = nc.dram_tensor("xfm", [F, ROWS], BF16, kind="ExternalInput")
    disALL_e = nc.dram_tensor("disALL", [128, C * W], F32, kind="ExternalInput")
    W1_e = nc.dram_tensor("W1sb", [F, H], BF16, kind="ExternalInput")
    W2_e = nc.dram_tensor("W2sb", [H, H], BF16, kind="ExternalInput")
    Wl_e = nc.dram_tensor("Wlsb", [H, 1], BF16, kind="ExternalInput")
    b1_e = nc.dram_tensor("b1sb", [128, H], F32, kind="ExternalInput")
    b2_e = nc.dram_tensor("b2sb", [128, H], F32, kind="ExternalInput")
    iota_e = nc.dram_tensor("iota128", [128, 128], BF16, kind="ExternalInput")
    iotaP_e = nc.dram_tensor("iotaP", [128, 384], F32, kind="ExternalInput")
    mask_e = nc.dram_tensor("mask48", [128, 1], F32, kind="ExternalInput")
    xown_e = nc.dram_tensor("xown", [F, NPAD], BF16, kind="ExternalInput")
    idx_e = nc.dram_tensor("idx16", [128, NCHUNK * 8], I16, kind="ExternalInput")
    dstloc_e = nc.dram_tensor("dstloc", [128, NCHUNK], BF16, kind="ExternalInput")
    disn_e = nc.dram_tensor("disn", [128, W], F32, kind="ExternalInput")
    glocal_e = nc.dram_tensor("glocal", [128, W], F32, kind="ExternalInput")
    out_e = nc.dram_tensor("out", [128, 3 * H], F32, kind="ExternalOutput")

    # internal DRAM
    PW = 3 * H
    shared = "Shared" if C > 4 else "Local"
    # [parity][half]: layer l gathers from parity (l-1)%2, AGs write l%2
    tblH = [[nc.dram_tensor(f"tblH{pb}_{h}", [TRh[h], 128], BF16,
                            addr_space=shared) for h in range(2)]
            for pb in range(2)]
    own_h = [nc.dram_tensor(f"own_h{h}", [WH[h] * 128, 128], BF16)
             for h in range(2)]

    with tile.TileContext(nc) as tc:
        from contextlib import ExitStack
        with ExitStack() as ctx:
            cpool = ctx.enter_context(tc.tile_pool(name="const", bufs=1))
            spool = ctx.enter_context(tc.tile_pool(name="s", bufs=4))
            mpool = ctx.enter_context(tc.tile_pool(name="msg", bufs=12))
            tpool = ctx.enter_context(tc.tile_pool(name="tmp", bufs=3))
            agg_ps = ctx.enter_context(
                tc.tile_pool(name="aggps", bufs=3, space="PSUM"))
            tp_ps = ctx.enter_context(
                tc.tile_pool(name="tpps", bufs=2, space="PSUM"))

            # ---- load constants into SBUF ----
            def load(name, ext, shape, dt):
                t = cpool.tile(shape, dt, tag=name)
                nc.sync.dma_start(t[:], ext.ap())
                return t

            W1sb = load("W1", W1_e, [F, H], BF16)
            W2sb = load("W2", W2_e, [H, H], BF16)
            Wlsb = load("Wl", Wl_e, [H, 1], BF16)
            b1sb = load("b1", b1_e, [128, H], F32)
            b2sb = load("b2", b2_e, [128, H], F32)
            iota = load("iota", iota_e, [128, 128], BF16)
            iotaP = load("iotaP", iotaP_e, [128, 384], F32)
            mask48 = load("mask48", mask_e, [128, 1], F32)
            idxsb = load("idx", idx_e, [128, NCHUNK * 8], I16)
            dstloc = load("dstloc", dstloc_e, [128, NCHUNK], BF16)
            disn = load("disn", disn_e, [128, W], F32)
            disALL = load("disALL", disALL_e, [128, C * W], F32)
            glocal = load("glocal", glocal_e, [128, W], F32)
            ident = cpool.tile([128, 128], BF16, tag="ident")
            make_identity(nc, ident[:])

            # persistent per-layer state tiles
            h_fm = [cpool.tile([H, NPAD], BF16, tag=f"hfm{i}", name=f"hfm{i}")
                    for i in range(2)]
            t_own = cpool.tile([128, W * H], BF16, tag="town")
            tpad = cpool.tile([128, W * 128], BF16, tag="tpad")
            nc.vector.memset(tpad[:], 0.0)
            h_nm = cpool.tile([128, W * H], BF16, tag="hnm")

            # zero the msg pool buffers once: trimmed gather calls skip their
            # trailing slots, so stale SBUF must be finite for the matmul
            for zi in range(12):
                mz = mpool.tile([128, CALLCH * H], BF16, tag="msg",
                                name=f"mz{zi}")
                nc.vector.memset(mz[:], 0.0)

            def sc_bc(t, lo, n, inner, pitch=None):
                a = t[:]
                step = a.ap[0][0]
                return _ap3(a, [[step, 128], [1, n], [0, inner]], a.offset + lo)

            # ---------------- conv transform helpers ----------------
            def transform_own_grp(h_src, Wsb, g):
                """own-node transform + t_own (dis*hw) + padded copy, windows
                of group g only."""
                wlo = g * WGRP
                nb = min(WGRP, W - wlo)
                ps = agg_ps.tile([128, WGRP * H], F32, tag="agg",
                                 name=f"tf{g}")
                for i in range(nb):
                    w = wlo + i
                    nc.tensor.matmul(
                        ps[:, i * H:(i + 1) * H],
                        lhsT=h_src[:, w * 128:(w + 1) * 128],
                        rhs=Wsb[:],
                        start=True, stop=True, skip_group_check=True)
                ps3 = _ap3(ps[:], [[ps[:].ap[0][0], 128], [H, nb], [1, H]])
                t3 = _ap3(t_own[:], [[t_own[:].ap[0][0], 128], [H, nb], [1, H]],
                          t_own[:].offset + wlo * H)
                nc.vector.tensor_tensor(
                    t3, ps3, sc_bc(disn, wlo, nb, H), op=ALU.mult)
                # spread compact -> padded (pad half is stale junk, never read)
                tp3 = _ap3(tpad[:], [[tpad[:].ap[0][0], 128], [128, nb], [1, H]],
                           tpad[:].offset + wlo * 128)
                to3 = _ap3(t_own[:], [[t_own[:].ap[0][0], 128], [H, nb], [1, H]],
                           t_own[:].offset + wlo * H)
                nc.vector.tensor_copy(tp3, to3)

            def transform_own(h_src, Wsb):
                for g in range(NG):
                    transform_own_grp(h_src, Wsb, g)

            def transform_all_conv1(xpool):
                """conv1: full-graph transform, half-major so layer-1
                gathers on half 0 can start while half 1 is still written."""
                NBAT = math.ceil(W / WGRP)
                for b in range(NBAT):
                    wlo = b * WGRP
                    nb = min(WGRP, W - wlo)
                    h = 1 if wlo >= WA else 0
                    for cblk in range(C):
                        xblk = xpool.tile([F, WGRP * 128], BF16, tag="xfm")
                        nc.sync.dma_start(
                            xblk[:, :nb * 128],
                            xfm_e.ap()[:, cblk * NPAD + wlo * 128:
                                       cblk * NPAD + (wlo + nb) * 128])
                        ps = agg_ps.tile([128, WGRP * H], F32, tag="agg")
                        for i in range(nb):
                            nc.tensor.matmul(
                                ps[:, i * H:(i + 1) * H],
                                lhsT=xblk[:, i * 128:(i + 1) * 128],
                                rhs=W1sb[:],
                                start=True, stop=True, skip_group_check=True)
                        pd = tpool.tile([128, WGRP * 128], BF16, tag="c1pad")
                        nc.vector.memset(pd[:], 0.0)
                        ps3 = _ap3(ps[:], [[ps[:].ap[0][0], 128], [H, nb], [1, H]])
                        pd3 = _ap3(pd[:], [[pd[:].ap[0][0], 128], [128, nb], [1, H]])
                        nc.vector.tensor_tensor(
                            pd3, ps3, sc_bc(disALL, cblk * W + wlo, nb, H),
                            op=ALU.mult)
                        # rows cblk*WH*128 + (w - half base)*128 + p
                        dr = _ap3(tblH[0][h].ap(),
                                  [[128, 128], [128 * 128, nb], [1, 128]],
                                  (cblk * WH[h] + wlo - h * WA) * 128 * 128)
                        nc.sync.dma_start(dr, pd[:, :nb * 128])

            def own_slice_grp(g):
                # group-g own rows (w - half base)*128 + p from tpad
                wlo = g * WGRP
                nb = min(WGRP, W - wlo)
                h = 1 if wlo >= WA else 0
                nc.sync.dma_start(
                    _ap3(own_h[h].ap(), [[128, 128], [128 * 128, nb],
                                         [1, 128]], (wlo - h * WA) * 128 * 128),
                    tpad[:, wlo * 128:(wlo + nb) * 128])

            def exchange_half(h, wb):
                nc.gpsimd.collective_compute(
                    "AllGather", ALU.bypass, replica_groups=rg,
                    ins=[own_h[h].ap().opt()],
                    outs=[tblH[wb][h].ap().opt()])

            # ---------------- aggregation (+ per-group drain) ----------------
            def aggregate_and_drain(layer, bsb, after_group=None):
                ncalls = 0
                psg = {}
                bank_start = {}
                glast = {}
                for ci in range(NCHUNK):
                    glast[int(meta["grp_of_chunk"][ci])] = ci
                drain_after = {v: k for k, v in glast.items()}
                qload = [0, 0, 0, 0]
                for ki, (sh, c_lo, c_hi) in enumerate(calls):
                    ncall = c_hi - c_lo
                    nidx = meta["call_nidx"][ki]
                    qsel = ncalls % 4
                    msg = mpool.tile([128, CALLCH * H], BF16, tag="msg")
                    tb = tblH[(layer - 1) % 2][sh]
                    in_ap = bass.AP(tb.ap().tensor, 0,
                                    [[128, TRh[sh]], [1, H]])
                    nch = (nidx + 127) // 128
                    dma_gather_raw(
                        nc,
                        out_ap=_ap3(msg[:], [[msg[:].ap[0][0], 128],
                                             [H, nch], [1, H]]),
                        in_ap=in_ap,
                        idxs_ap=idxsb[:, c_lo * 8:c_lo * 8 + nch * 8],
                        num_idxs=nidx,
                        elem_size=H,
                        elem_step=128,
                        queue_num=qsel)
                    qload[qsel] += nidx
                    ncalls += 1
                    Stile = spool.tile([128, CALLCH * 128], BF16, tag="S")
                    S3 = _ap3(Stile[:], [[Stile[:].ap[0][0], 128],
                                         [128, ncall], [1, 128]])
                    iota_bc = _ap3(iota[:], [[iota[:].ap[0][0], 128],
                                             [0, ncall], [1, 128]])
                    nc.vector.tensor_tensor(
                        S3, iota_bc, sc_bc(dstloc, c_lo, ncall, 128),
                        op=ALU.is_equal)
                    for j in range(ncall):
                        ci = c_lo + j
                        w = int(chunk_w[ci])
                        g = w // WGRP
                        if g not in psg:
                            psg[g] = agg_ps.tile(
                                [128, WGRP * H], F32, tag="agg",
                                name=f"agg_l{layer}_g{g}")
                        wl = w - g * WGRP
                        rhs = msg[:, j * H:(j + 1) * H]
                        mm = nc.tensor.matmul(
                            psg[g][:, wl * H:(wl + 1) * H],
                            lhsT=Stile[:, j * 128:(j + 1) * 128],
                            rhs=rhs,
                            start=bool(start_f[ci]),
                            stop=bool(stop_f[ci]),
                            skip_group_check=True)
                        bk = int(meta["bank_of_chunk"][ci])
                        if start_f[ci]:
                            bank_start[bk] = mm
                        elif bk in bank_start:
                            bass._add_dep_helper(
                                mm.ins, bank_start[bk].ins, sync=False,
                                reason="psum zero-region order")
                        if ci in drain_after:
                            gdone = drain_after[ci]
                            drain_group(gdone, psg.pop(gdone), bsb,
                                        layer=layer)
                            if after_group is not None:
                                after_group(gdone)

            def drain_group(g, ps, bsb, layer=0):
                    wlo = g * WGRP
                    nb = min(WGRP, W - wlo)
                    pstep = ps[:].ap[0][0]
                    ps3 = _ap3(ps[:], [[pstep, 128], [H, nb], [1, H]])
                    tmp = tpool.tile([128, WGRP * H], F32, tag="dr")
                    ts = tmp[:].ap[0][0]
                    tmp3 = _ap3(tmp[:], [[ts, 128], [H, nb], [1, H]])
                    to3 = _ap3(t_own[:], [[t_own[:].ap[0][0], 128], [H, nb], [1, H]],
                               t_own[:].offset + wlo * H)
                    # agg + t_own
                    nc.vector.tensor_tensor(tmp3, ps3, to3, op=ALU.add)
                    # * dis
                    nc.vector.tensor_tensor(
                        tmp3, tmp3, sc_bc(disn, wlo, nb, H), op=ALU.mult)
                    # + bias
                    bb = _ap3(bsb[:], [[bsb[:].ap[0][0], 128], [0, nb],
                                       [1, H]], 0)
                    nc.vector.tensor_tensor(tmp3, tmp3, bb, op=ALU.add)
                    # relu -> bf16 node-major
                    hn3 = _ap3(h_nm[:], [[h_nm[:].ap[0][0], 128], [H, nb], [1, H]],
                               h_nm[:].offset + wlo * H)
                    nc.vector.tensor_scalar(
                        hn3, tmp3, 0.0, None, op0=ALU.max)
                    # mask tail-window pads
                    if g == NG - 1 and meta["tailp"] < 128:
                        lastw = W - 1
                        hl = _ap3(h_nm[:], [[h_nm[:].ap[0][0], 128], [1, H]],
                                  h_nm[:].offset + lastw * H)
                        mb = _ap3(mask48[:],
                                  [[mask48[:].ap[0][0], 128], [0, H]], 0)
                        nc.vector.tensor_tensor(hl, hl, mb, op=ALU.mult)

            def to_fm_grp(dst_fm, g):
                for w in range(g * WGRP, min((g + 1) * WGRP, W)):
                    tp = tp_ps.tile([H, 128], BF16, tag="tp")
                    nc.tensor.transpose(
                        out=tp[:],
                        in_=_ap3(h_nm[:], [[h_nm[:].ap[0][0], 128], [1, H]],
                                 h_nm[:].offset + w * H),
                        identity=ident[:])
                    nc.scalar.copy(dst_fm[:, w * 128:(w + 1) * 128], tp[:])

            # ================= layer schedule =================
            # conv1 table: transform own nodes only, then the same per-half
            # AllGather path as every other layer (parity 0)
            with tc.tile_pool(name="xfm", bufs=2) as xpool:
                xo = xpool.tile([F, NPAD], BF16, tag="xfm", name="xo")
                nc.sync.dma_start(xo[:], xown_e.ap())
                for g in range(NG):
                    transform_own_grp(xo, W1sb, g)
                    own_slice_grp(g)
                    if g == 1:
                        exchange_half(0, 0)
                    elif g == NG - 1:
                        exchange_half(1, 0)

            pool_state = {}

            def pooling_grp(g):
                wlo = g * WGRP
                nwg = min(WGRP, W - wlo)
                pps = pool_state["pps"]
                SGpool = pool_state["sgpool"]
                for blk in range(3):
                    for w0 in range(wlo, wlo + nwg, 8):
                        nb = min(8, wlo + nwg - w0)
                        SG = SGpool.tile([128, 8 * 128], BF16, tag="SG")
                        iob = _ap3(iotaP[:], [[iotaP[:].ap[0][0], 128],
                                              [0, nb], [1, 128]], blk * 128)
                        nc.vector.tensor_tensor(
                            _ap3(SG[:], [[SG[:].ap[0][0], 128], [128, nb],
                                         [1, 128]]),
                            iob, sc_bc(glocal, w0, nb, 128), op=ALU.is_equal)
                        for i in range(nb):
                            w = w0 + i
                            mm = nc.tensor.matmul(
                                pps[:, blk * H:(blk + 1) * H],
                                lhsT=SG[:, i * 128:(i + 1) * 128],
                                rhs=_ap3(h_nm[:],
                                         [[h_nm[:].ap[0][0], 128], [1, H]],
                                         h_nm[:].offset + w * H),
                                start=(blk == 0 and w == 0),
                                stop=(blk == 2 and w == W - 1),
                                skip_group_check=True)
                            if blk == 0 and w == 0:
                                pool_state["start"] = mm
                            else:
                                bass._add_dep_helper(
                                    mm.ins, pool_state["start"].ins,
                                    sync=False,
                                    reason="psum zero-region order")

            for l in range(1, 6):
                bsb = b1sb if l == 1 else b2sb
                if l < 5:
                    hf_next = h_fm[(l + 1) % 2]

                    def after_group(g, hf=hf_next, wb=l % 2):
                        to_fm_grp(hf, g)
                        transform_own_grp(hf, W2sb, g)
                        own_slice_grp(g)
                        if g == 1:
                            exchange_half(0, wb)
                        elif g == NG - 1:
                            exchange_half(1, wb)
                else:
                    pool_state["pps"] = agg_ps.tile(
                        [128, WGRP * H], F32, tag="agg", name="pps")
                    pool_state["sgpool"] = spool

                    def after_group(g):
                        pooling_grp(g)
                aggregate_and_drain(l, bsb, after_group=after_group)

            ppsb = tpool.tile([128, PW], F32, tag="ppsb")
            nc.vector.tensor_copy(ppsb[:], pool_state["pps"][:, :PW])
            nc.sync.dma_start(out_e.ap(), ppsb[:])


# ---------------------------------------------------------------------------
# Entry points
# ---------------------------------------------------------------------------

def run(inputs, C=8, G=1000, trace=False):
    meta, in_maps = prep(
        inputs["x"], inputs["W1"], inputs["b1"], inputs["W2"], inputs["b2"],
        inputs["Wl"], inputs["bl"], inputs["edge_index"], inputs["batch"],
        C=C, G=G)
    nc = bacc.Bacc("TRN2", target_bir_lowering=False, debug=False,
                   num_devices=C, num_swdge_queues=4)
    build(nc, meta)
    nc.compile()
    from concourse.bass_utils import run_bass_kernel_spmd
    res = run_bass_kernel_spmd(nc, in_maps, core_ids=list(range(C)),
                               trace=trace)
    parts = [res.results[c]["out"] for c in range(C)]
    out = host_finish(meta, parts, inputs, C, G)
    return out, res


def host_finish(meta, parts, inputs, C, G):
    """Combine per-core pooled partial sums, divide by counts, final linear."""
    H = meta["H"]
    pooled = np.zeros(((meta["BLK"] + 3) * 128, H), np.float32)
    for c in range(C):
        part = np.asarray(parts[c], np.float32)   # [128, 3H]
        base = meta["wk"][c] * 128
        for b in range(3):
            pooled[base + b * 128: base + (b + 1) * 128] += \
                part[:, b * H:(b + 1) * H]
    counts = np.bincount(np.asarray(inputs["batch"], np.int64),
                         minlength=G).astype(np.float32)
    pooledG = pooled[:G] / np.maximum(counts, 1.0)[:, None]
    Wl = np.asarray(inputs["Wl"], np.float32).reshape(H, -1)
    bl = np.asarray(inputs["bl"], np.float32)
    return (pooledG @ Wl + bl).astype(np.float32)


def kernel(**inputs):
    out, _ = run(inputs)
    return out
